# revision 35
# baseline (speedup 1.0000x reference)
"""Grouped (MoE-style) linear on 8 trn2 NeuronCores.

out[t] = hidden_states[t] @ weight[g(t)], where token t belongs to group g iff
offsets[g-1] <= t < offsets[g] (searchsorted right semantics; tokens at or past
offsets[-1] get zero output).

Strategy: expert-parallel. Core g owns weight[g] and the contiguous token run
of group g. Routing is done host-side (offsets are host data), each core runs
an identical Bass program tiled as 128-token blocks, contraction in 8 chunks
of 128, PSUM-accumulated. Inputs are cast to bf16 on the host (PSUM stays
fp32): same 1 col/cycle PE rate as fp32r, half the input DMA. Measured
rel-max error ~2.3e-3 for this distribution (gate 2e-2).

Schedule, driven by trace analysis. Hard constraints observed on hardware:
- Data DMA cannot start before ~8.5us (fixed framework preamble + DGE spin
  up). Aggregate DMA is row-size-bound (~300KB/us at 1-2KB rows); per-queue
  throughput is roughly aggregate/active-queues. Three DGE queues exist:
  sync (SP), scalar (Act), gpsimd (SWDGE).
- The tensor engine clocks 1.2GHz until ~3us of continuous work, then
  2.4GHz, and idle gaps >~0.5us reset it. Dummy warmup matmuls on a memset
  tile ramp the clock inside the DMA-startup shadow; the schedule afterwards
  must be gap-free.
- Tile's PSUM bank tracker serializes ANY same-tensor access pair (even two
  reads), so parallel copies need separate PSUM tensors.

Gap-free schedule: phase 1a runs token blocks 0..7 k-OUTER on output cols
0:512 only (8 open PSUM banks), consuming one 128KB W half-chunk plus one
256KB k-major x chunk per 1.73us round, laid out across the three queues
(measured: sync starts ~8.0us, scalar ~8.4us and slower, gpsimd ~8.8us with
+1.5us completion-sem latency; each queue pays ~1.5-2us per-item handoff) so
every chunk lands before its round. Phase 1b repeats for cols 512:1024; its
W halves stream as paired items on sync+scalar during 1a, arriving ~5us
early. Phase 2 runs blocks 8..15 tile-major, fully resident, output halves
split across queues. The last block runs in three column strips
(512/256/256) with cascaded flushes so only a 128KB quarter-tile drain
trails the final matmul.

Host packing:
  xtA[k, p, tb*128+tok] = X_g[tb*128 + tok, k*128 + p]   (blocks 0..7)
  xtB[tb, p, k, tok]    = X_g[(8+tb)*128 + tok, k*128 + p]
  w[p, k, n]            = W_g[k*128 + p, n]
"""
import ml_dtypes
import numpy as np

import concourse.bass as bass
import concourse.tile as tile
from concourse import bacc, mybir
from concourse.bass_utils import run_bass_kernel_spmd

GROUPS = 8
TOKENS = 16384
IN_F = 1024
OUT_F = 1024
KCH = IN_F // 128  # contraction chunks
NWARM = 22         # dummy ramp matmuls: span the startup until round-0 data
PH1 = 8            # token blocks in the k-outer phase (= PSUM banks)


def build(ntb: int) -> bass.Bass:
    """One core's program: ntb 128-token blocks through a 1024x1024 expert."""
    f32 = mybir.dt.float32
    bf16 = mybir.dt.bfloat16
    nc = bacc.Bacc()
    p1 = min(PH1, ntb)
    nb2 = ntb - p1
    xta_d = nc.dram_tensor("xta", [KCH, 128, p1 * 128], bf16, kind="ExternalInput")
    if nb2:
        xtb_d = nc.dram_tensor("xtb", [nb2, 128, KCH, 128], bf16,
                               kind="ExternalInput")
    w_d = nc.dram_tensor("w", [128, KCH, OUT_F], bf16, kind="ExternalInput")
    out_d = nc.dram_tensor("out", [ntb * 128, OUT_F], f32, kind="ExternalOutput")

    h0 = min(4, p1)  # k=0 x chunk split point (lands on two queues in parallel)

    with tile.TileContext(nc) as tc:
        with (
            tc.tile_pool(name="wp", bufs=1) as wp,
            tc.tile_pool(name="xp", bufs=max(1, nb2)) as xp,
            tc.tile_pool(name="op", bufs=4) as op,
            tc.tile_pool(name="fp", bufs=1) as fp,
            tc.tile_pool(name="ps", bufs=8, space="PSUM") as psp,
        ):
            # PE p-state ramp tile (no DMA dependency).
            dummy = fp.tile([128, 256], bf16, tag="warm")
            nc.gpsimd.memset(dummy[:], 0)

            wt = wp.tile([128, KCH, OUT_F], bf16)
            xta = fp.tile([128, KCH, p1, 128], bf16, tag="xta")
            # Input streams laid out per measured queue physics (sync starts
            # ~8.0us, scalar ~8.4us and slower, gpsimd ~8.8us with +1.5us
            # completion-sem latency) so every phase-1a round's chunk lands
            # before the PE needs it.
            # sync queue, in order:
            nc.sync.dma_start(out=xta[:, 0, 0:h0], in_=xta_d[0, :, 0:h0 * 128])
            for k in [0, 1]:
                nc.sync.dma_start(out=wt[:, k, 0:512], in_=w_d[:, k, 0:512])
            for k in range(2, KCH, 2):
                nc.sync.dma_start(out=xta[:, k], in_=xta_d[k])
            nc.sync.dma_start(out=wt[:, 6, 0:512], in_=w_d[:, 6, 0:512])
            # scalar queue, in order:
            if h0 < p1:
                nc.scalar.dma_start(out=xta[:, 0, h0:], in_=xta_d[0, :, h0 * 128:])
            for k in [2, 3, 4, 5, 7]:
                nc.scalar.dma_start(out=wt[:, k, 0:512], in_=w_d[:, k, 0:512])
            # W cols 512:1024 (phase 1b) as paired 256KB items split across
            # sync and scalar -- both idle after their phase-1a items, so
            # every pair lands >=5us before its 1b round.
            for k in range(0, KCH, 2):
                eng = nc.sync if k < KCH // 2 else nc.scalar
                eng.dma_start(out=wt[:, k:k + 2, 512:1024],
                              in_=w_d[:, k:k + 2, 512:1024])
            # gpsimd queue (SWDGE; ~1.5us completion latency, never
            # just-in-time): odd x chunks.
            for k in range(1, KCH, 2):
                nc.gpsimd.dma_start(out=xta[:, k], in_=xta_d[k])
            xts = []
            for t in range(nb2):
                xtn = xp.tile([128, KCH, 128], bf16, tag="xt", name=f"xt{t}")
                eng = nc.sync if t % 2 == 0 else nc.gpsimd
                eng.dma_start(out=xtn[:], in_=xtb_d[t])
                xts.append(xtn)

            pa = {}
            pb = {}
            for tb in range(p1):
                pa[tb] = psp.tile([128, 512], f32, tag="acc", name=f"pa{tb}")

            for _ in range(NWARM):
                nc.tensor.matmul(pa[0][:, 0:256], dummy[:, 0:128], dummy[:],
                                 start=True, stop=True, skip_group_check=True)

            def flush_half(tb, p, lo, on_scalar):
                """Copy one [128,512] psum half; scalar-copied halves DMA on
                scalar's queue, vector-copied ones on gpsimd's."""
                oh = op.tile([128, 512], f32, tag="oh", name=f"oh{tb}_{lo}")
                if on_scalar:
                    nc.scalar.copy(oh[:], p[:])
                    nc.scalar.dma_start(
                        out=out_d[tb * 128:(tb + 1) * 128, lo:lo + 512], in_=oh[:])
                else:
                    nc.vector.tensor_copy(oh[:], p[:])
                    nc.gpsimd.dma_start(
                        out=out_d[tb * 128:(tb + 1) * 128, lo:lo + 512], in_=oh[:])

            def flush_final(tb, p):
                """Last block, cols 512:1024 in one [128,512] psum tensor
                (ntb<=8 fallback): serialized copies, two-queue DMA."""
                ota = fp.tile([128, 256], f32, tag="ota")
                otb = fp.tile([128, 256], f32, tag="otb")
                nc.scalar.copy(ota[:], p[:, 0:256])
                nc.sync.dma_start(out=out_d[tb * 128:(tb + 1) * 128, 512:768],
                                  in_=ota[:])
                nc.vector.tensor_copy(otb[:], p[:, 256:512])
                nc.scalar.dma_start(out=out_d[tb * 128:(tb + 1) * 128, 768:1024],
                                    in_=otb[:])

            # Phase 1a: k-outer, blocks 0..p1, cols 0:512. A couple of
            # insurance dummies after round 0 dice any residual arrival-
            # jitter gap (dummy is all-zero, so mid-group accumulation is a
            # no-op on pa[0]'s value).
            for k in range(KCH):
                for tb in range(p1):
                    nc.tensor.matmul(pa[tb][:], xta[:, k, tb, :], wt[:, k, 0:512],
                                     start=(k == 0), stop=(k == KCH - 1))
                if k == 0:
                    for _ in range(2):
                        nc.tensor.matmul(pa[0][:, 0:256], dummy[:, 0:128],
                                         dummy[:], start=False, stop=False,
                                         skip_group_check=True)
            for tb in range(p1):
                flush_half(tb, pa[tb], 0, on_scalar=(tb % 2 == 0))

            # Phase 1b: same blocks, cols 512:1024. Insurance dummies after
            # the first rounds dice the psum-slot/W-arrival jitter gap.
            for tb in range(p1):
                pb[tb] = psp.tile([128, 512], f32, tag="acc", name=f"pb{tb}")
            for k in range(KCH):
                for tb in range(p1):
                    nc.tensor.matmul(pb[tb][:], xta[:, k, tb, :], wt[:, k, 512:1024],
                                     start=(k == 0), stop=(k == KCH - 1))
                for _ in range({0: 2, 1: 1}.get(k, 0)):
                    nc.tensor.matmul(pb[0][:, 0:256], dummy[:, 0:128],
                                     dummy[:], start=False, stop=False,
                                     skip_group_check=True)
            for tb in range(p1):
                if tb == ntb - 1:
                    flush_final(tb, pb[tb])
                else:
                    flush_half(tb, pb[tb], 512, on_scalar=(tb % 2 == 0))

            # Phase 2: tile-major, everything resident.
            for t in range(nb2):
                tb = p1 + t
                if tb == ntb - 1:
                    # Column strips 0:512, 512:768, 768:1024 with cascaded
                    # flushes; separate psum tensors so the copies (scalar,
                    # scalar, vector) never serialize against each other.
                    a = psp.tile([128, 512], f32, tag="acc", name=f"a{tb}")
                    blo = psp.tile([128, 256], f32, tag="acc", name="blo")
                    bhi = psp.tile([128, 256], f32, tag="acc", name="bhi")
                    for k in range(KCH):
                        nc.tensor.matmul(a[:], xts[t][:, k, :], wt[:, k, 0:512],
                                         start=(k == 0), stop=(k == KCH - 1))
                    flush_half(tb, a, 0, on_scalar=True)
                    for k in range(KCH):
                        nc.tensor.matmul(blo[:], xts[t][:, k, :], wt[:, k, 512:768],
                                         start=(k == 0), stop=(k == KCH - 1))
                    olo = fp.tile([128, 256], f32, tag="ota")
                    nc.scalar.copy(olo[:], blo[:])
                    nc.sync.dma_start(out=out_d[tb * 128:(tb + 1) * 128, 512:768],
                                      in_=olo[:])
                    for k in range(KCH):
                        nc.tensor.matmul(bhi[:], xts[t][:, k, :], wt[:, k, 768:1024],
                                         start=(k == 0), stop=(k == KCH - 1))
                    ohi = fp.tile([128, 256], f32, tag="otb")
                    nc.vector.tensor_copy(ohi[:], bhi[:])
                    nc.scalar.dma_start(out=out_d[tb * 128:(tb + 1) * 128, 768:1024],
                                        in_=ohi[:])
                else:
                    a = psp.tile([128, 512], f32, tag="acc", name=f"a{tb}")
                    b = psp.tile([128, 512], f32, tag="acc", name=f"b{tb}")
                    for k in range(KCH):
                        nc.tensor.matmul(a[:], xts[t][:, k, :], wt[:, k, 0:512],
                                         start=(k == 0), stop=(k == KCH - 1))
                        nc.tensor.matmul(b[:], xts[t][:, k, :], wt[:, k, 512:1024],
                                         start=(k == 0), stop=(k == KCH - 1))
                    # Halves on separate queues: a 512KB blob on one queue
                    # takes ~5us to drain; 256KB halves on two don't. The
                    # last tiles' b-halves go on sync (idle late, no SWDGE
                    # completion latency in the epilogue drain).
                    flush_half(tb, a, 0, on_scalar=True)
                    oh = op.tile([128, 512], f32, tag="oh", name=f"ohb{tb}")
                    nc.vector.tensor_copy(oh[:], b[:])
                    eng = nc.sync if tb >= ntb - 3 else nc.gpsimd
                    eng.dma_start(out=out_d[tb * 128:(tb + 1) * 128, 512:1024],
                                  in_=oh[:])
    nc.compile()
    return nc


def _pack_core(x_slice: np.ndarray, w_g: np.ndarray, ntb: int):
    n = x_slice.shape[0]
    p1 = min(PH1, ntb)
    xp = np.zeros((ntb * 128, IN_F), dtype=np.float32)
    xp[:n] = x_slice
    xa = np.ascontiguousarray(
        xp[:p1 * 128].reshape(p1, 128, KCH, 128).transpose(2, 3, 0, 1)
        .reshape(KCH, 128, p1 * 128).astype(ml_dtypes.bfloat16)
    )
    m = {"xta": xa}
    if ntb > p1:
        m["xtb"] = np.ascontiguousarray(
            xp[p1 * 128:].reshape(ntb - p1, 128, KCH, 128).transpose(0, 3, 2, 1)
            .astype(ml_dtypes.bfloat16)
        )
    m["w"] = np.ascontiguousarray(
        w_g.reshape(KCH, 128, OUT_F).transpose(1, 0, 2).astype(ml_dtypes.bfloat16)
    )
    return m


def kernel(hidden_states: np.ndarray, weight: np.ndarray, offsets: np.ndarray,
           _trace: bool = False):
    hs = np.ascontiguousarray(hidden_states, dtype=np.float32)
    w = np.ascontiguousarray(weight, dtype=np.float32)
    off = np.asarray(offsets).astype(np.int64)

    ends = np.clip(off, 0, TOKENS)
    starts = np.concatenate(([0], ends[:-1]))
    starts = np.minimum(starts, ends)
    ns = ends - starts

    ntb = max(1, int(-(-ns.max() // 128)))
    nc = build(ntb)

    in_maps = [
        _pack_core(hs[starts[g]:ends[g]], w[g], ntb) for g in range(GROUPS)
    ]

    res = run_bass_kernel_spmd(nc, in_maps, list(range(GROUPS)), trace=_trace)

    out = np.zeros((TOKENS, OUT_F), dtype=np.float32)
    for g in range(GROUPS):
        if ns[g] > 0:
            out[starts[g]:ends[g]] = res.results[g]["out"][:ns[g]]
    if _trace:
        return out, res
    return out
